# revision 10
# baseline (speedup 1.0000x reference)
"""Causal self-attention Bass/TRN2 kernel (v2, bf16 compute).

Shapes (hardcoded): query [2, 2048, 1024], 16 heads, d=64.
Sharding: 8 cores = 2 batches x 4 head-groups (4 heads per core, tensor
parallel on the QKV/proj weight columns). Each core computes a partial
out projection out_t = Wp_slice^T @ y^T (shape [1024, 2048] f32); host sums
the 4 partials per batch, transposes, and adds the folded bias.

Host-side exact simplifications:
  * x is pre-transposed per batch (x^T [1024, 2048]) and cast to bf16, so the
    device never runs PE transposes.
  * bk is dropped: q . bk is constant along the softmax axis (shift
    invariance), so it never affects the output.
  * bv is folded into the output bias: y = P@(x Wv) + (P@1) bv^T and softmax
    rows sum to 1 after normalization, so out += bv @ Wp, added to bp on host.
  * bq is applied on device (fused into the Q PSUM->SBUF activation copy).

Per-core pipeline (all matmuls bf16, f32 PSUM accumulate):
  B(m,g): Q^T/K^T [128, 512]-chunk projections (8 k-matmuls each) + ACT copy
          (Q with bq bias) -> qt/kt bf16 [128, 2, 2048].
  C(it):  V natural [128, 256] (8 k-matmuls) + DVE copy into va bf16
          [128, h, it, 65]; column 64 is memset to 1 so the M=65 PV matmul
          also produces the softmax denominator row.
  D(hp,g): per 128-row K-block j: S^T for both heads of the pair into one
          [128, 1024] PSUM tile; additive -1e30 causal mask on the diagonal
          128-blocks (DVE); ACT exp (scale=1/8, no max subtraction -- scores
          are bounded for this problem) -> p12 bf16; PV accumulate into
          yd1/yd2 [65, 512] PSUM. Emission is software-pipelined 2 blocks
          ahead so ACT exp latency never stalls the PE. Normalization:
          DVE reciprocal of the denominator row, Pool partition_broadcast,
          DVE multiply -> yt bf16 (head 1 of the pair lands at partitions
          64:128 via a small SBUF->SBUF shift DMA).
  E(g):   out_t chunk = Wp^T y^T, staged PSUM->SBUF on alternating DVE/ACT,
          DMA out f32.
Schedule interleaves B/C/E between D chunks to keep the PE saturated while
the ACT engine drains the exp backlog.

This walrus build accepts only ONE sync-wait command per TPB instruction, so
after Tile scheduling we hoist excess waits into standalone InstEventSemaphore
instructions (split_excess_waits).
"""

import numpy as np
import ml_dtypes

import concourse.bass as bass
import concourse.mybir as mybir
import concourse.tile as tile
from concourse.bass_utils import run_bass_kernel_spmd

B, T, C, H = 2, 2048, 1024, 16
D = C // H            # 64 head dim
HC = 4                # heads per core
DC = HC * D           # 256 dcols per core
KT = C // 128         # 8 contraction tiles
NT = T // 128         # 16 t-tiles
TCH = T // 512        # 4 t-chunks of 512
SCALE = 1.0 / np.sqrt(D)
FILL_EVERY = 3
NEG = -1.0e30

f32 = mybir.dt.float32
f32r = mybir.dt.float32r
bf16 = mybir.dt.bfloat16
BF = ml_dtypes.bfloat16

_CACHE = {}


def _split_excess_waits(nc, max_inline=1):
    """Hoist excess per-instruction waits into standalone event-sem waits."""
    n = 0
    for f in nc.m.functions:
        for bb in f.blocks:
            new_insts = []
            for inst in bb.instructions:
                si = inst.sync_info
                waits = list(si.on_wait) if (si is not None and si.on_wait) else []
                if len(waits) > max_inline:
                    hoist, keep = waits[:-max_inline], waits[-max_inline:]
                    for w in hoist:
                        ev = mybir.InstEventSemaphore(
                            name=nc.get_next_instruction_name(),
                            engine=inst.engine,
                            ins=[],
                            outs=[],
                            sync_info=mybir.SyncInfo(on_wait=[w], on_update=[]),
                        )
                        nc.register_instruction(ev, overwrite=True)
                        new_insts.append(ev)
                        n += 1
                    si.on_wait = keep
                new_insts.append(inst)
            bb.instructions[:] = new_insts
    return n


def _make_diag_mask(nc, mask):
    """mask[p, f] = 0 where f >= p (valid, t>=s) else -1e30."""
    nc.gpsimd.memset(mask, 0.0)
    nc.gpsimd.affine_select(
        out=mask, in_=mask, compare_op=mybir.AluOpType.is_ge,
        fill=NEG, base=0, pattern=[[1, 128]], channel_multiplier=-1,
    )


def _build_program():
    nc = bass.Bass("TRN2", target_bir_lowering=False, debug=False)

    xt_d = nc.dram_tensor("xt", [C, T], bf16, kind="ExternalInput").ap()
    wq_d = nc.dram_tensor("wq", [C, DC], bf16, kind="ExternalInput").ap()
    wk_d = nc.dram_tensor("wk", [C, DC], bf16, kind="ExternalInput").ap()
    wv_d = nc.dram_tensor("wv", [C, DC], bf16, kind="ExternalInput").ap()
    wp_d = nc.dram_tensor("wp", [DC, C], bf16, kind="ExternalInput").ap()
    bq_d = nc.dram_tensor("bq", [DC], f32, kind="ExternalInput").ap()
    out_d = nc.dram_tensor("out_t", [C, T], bf16, kind="ExternalOutput").ap()

    ident_fn = mybir.ActivationFunctionType.Identity

    with (
        tile.TileContext(nc) as tc,
        nc.allow_low_precision("bf16 compute fits the 2e-2 rel tolerance"),
    ):
        with (
            tc.tile_pool(name="const", bufs=1) as cpool,
            tc.tile_pool(name="big", bufs=1) as big,
            tc.tile_pool(name="pp", bufs=4) as pp,
            tc.tile_pool(name="rp", bufs=2) as rp,
            tc.tile_pool(name="rbp", bufs=2) as rbp,
            tc.tile_pool(name="ytp", bufs=2) as ytp,
            tc.tile_pool(name="obp", bufs=3) as obp,
            tc.tile_pool(name="ps_ay", bufs=2, space="PSUM") as ps_ay,
            tc.tile_pool(name="ps_y", bufs=2, space="PSUM") as ps_y,
            tc.tile_pool(name="ps_s", bufs=2, space="PSUM") as ps_s,
        ):
            # PSUM budget (8 banks): acc rotation 2 (qp/kp/vp/op),
            # yd rotation 2 (yd1+yd2, decoupled so the normalize chain never
            # blocks projection tiles), s12 2 x 2 banks.
            def acc_tile():
                return ps_ay.tile([128, 512], f32, name="acc")

            def yd_tile():
                return ps_y.tile([128, 512], f32, name="yd")
            # ---- constants ----
            # tri01[p, f] = 1 where f >= p (valid, t>=s) else 0; applied
            # multiplicatively to exp(S) on the Pool engine (Pool cannot
            # access PSUM, but p12 lives in SBUF)
            tri01 = cpool.tile([128, 128], bf16)
            nc.gpsimd.memset(tri01, 1.0)
            nc.gpsimd.affine_select(
                out=tri01, in_=tri01, compare_op=mybir.AluOpType.is_ge,
                fill=0.0, base=0, pattern=[[1, 128]], channel_multiplier=-1,
            )
            bq_sb = cpool.tile([128, 2, 1], f32)
            ones_f = cpool.tile([128, 64], f32)
            nc.gpsimd.memset(ones_f, 1.0)
            ones_sb = ones_f.bitcast(f32r)

            # ---- persistent big tensors ----
            xt = big.tile([128, KT, T], bf16)      # X^T
            wq_sb = big.tile([128, KT, DC], bf16)
            wk_sb = big.tile([128, KT, DC], bf16)
            wv_sb = big.tile([128, KT, DC], bf16)
            wp_sb = big.tile([128, 2, C], bf16)
            qt = big.tile([128, 2, T], bf16)       # Q^T [dcol, t]
            kt = big.tile([128, 2, T], bf16)       # K^T
            # V augmented per head: [s, 65] = [V_h | ones]; the M=65 PV matmul
            # computes y rows 0..63 and the softmax denominator row 64.
            va = big.tile([128, HC, NT, 65], bf16)
            yt = big.tile([128, 2, T], bf16)       # normalized y^T

            nc.gpsimd.memset(va[:, :, :, 64:65], 1.0)

            # ---- input DMAs, ordered for earliest PE start ----
            nc.sync.dma_start(
                out=bq_sb, in_=bq_d.rearrange("(m p o) -> p m o", p=128, o=1))
            for k in range(KT):
                nc.sync.dma_start(
                    out=wq_sb[:, k, :], in_=wq_d[bass.ts(k, 128), :])
                nc.sync.dma_start(
                    out=xt[:, k, 0:512], in_=xt_d[bass.ts(k, 128), 0:512])
            nc.sync.dma_start(
                out=wv_sb, in_=wv_d.rearrange("(k p) d -> p k d", p=128))
            nc.sync.dma_start(
                out=wk_sb, in_=wk_d.rearrange("(k p) d -> p k d", p=128))
            for g in range(1, TCH):
                nc.sync.dma_start(
                    out=xt[:, :, bass.ts(g, 512)],
                    in_=xt_d[:, bass.ts(g, 512)].rearrange(
                        "(k p) t -> p k t", p=128))
            nc.sync.dma_start(
                out=wp_sb, in_=wp_d.rearrange("(m p) c -> p m c", p=128))

            # ---- stage helpers ----
            def proj_qk(m, g):
                ts_g = bass.ts(g, 512)
                qp = acc_tile()
                for k in range(KT):
                    nc.tensor.matmul(
                        qp, wq_sb[:, k, bass.ts(m, 128)], xt[:, k, ts_g],
                        start=(k == 0), stop=(k == KT - 1),
                    )
                nc.scalar.activation(
                    out=qt[:, m, ts_g], in_=qp, func=ident_fn,
                    bias=bq_sb[:, m, :], scale=1.0,
                )
                kp = acc_tile()
                for k in range(KT):
                    nc.tensor.matmul(
                        kp, wk_sb[:, k, bass.ts(m, 128)], xt[:, k, ts_g],
                        start=(k == 0), stop=(k == KT - 1),
                    )
                nc.scalar.copy(out=kt[:, m, ts_g], in_=kp)

            def proj_v(it):
                # full-bank allocation (use first DC cols) to avoid
                # intra-bank PE-write / DVE-read overlap
                vp_full = acc_tile()
                vp = vp_full[:, 0:DC]
                for k in range(KT):
                    nc.tensor.matmul(
                        vp, xt[:, k, bass.ts(it, 128)], wv_sb[:, k, :],
                        start=(k == 0), stop=(k == KT - 1),
                    )
                nc.vector.tensor_copy(
                    out=va[:, :, it, 0:64],
                    in_=vp.rearrange("p (h d) -> p h d", h=HC),
                )

            def attn(hp, g, pull=None):
                nj = 4 * g + 4
                yd1 = yd_tile()
                yd2 = yd_tile()

                def emit_s(j):
                    r = j - 4 * g
                    lo = 128 * r if r > 0 else 0
                    w = 512 - lo
                    s12 = ps_s.tile([128, 1024], f32, name="s12")
                    tsl = bass.ds(512 * g + lo, w)
                    nc.tensor.matmul(
                        s12[:, lo:512], kt[0:64, hp, bass.ts(j, 128)],
                        qt[0:64, hp, tsl], start=True, stop=True,
                    )
                    nc.tensor.matmul(
                        s12[:, 512 + lo:1024], kt[64:128, hp, bass.ts(j, 128)],
                        qt[64:128, hp, tsl], start=True, stop=True,
                    )
                    p12 = pp.tile([128, 1024], bf16, name="p12")
                    sv = s12.rearrange("p (h t) -> p h t", h=2)[:, :, lo:]
                    pv_ = p12.rearrange("p (h t) -> p h t", h=2)[:, :, lo:]
                    nc.scalar.activation(
                        out=pv_, in_=sv,
                        func=mybir.ActivationFunctionType.Exp,
                        scale=float(SCALE),
                    )
                    if r >= 0:
                        # zero the upper triangle of the diagonal block after
                        # exp, on the otherwise-idle Pool engine (unmasked
                        # scores are bounded, so exp cannot overflow; the
                        # denominator is formed from the masked p12 by PV)
                        nc.gpsimd.tensor_mul(
                            p12[:, lo:lo + 128], p12[:, lo:lo + 128], tri01)
                        nc.gpsimd.tensor_mul(
                            p12[:, 512 + lo:512 + lo + 128],
                            p12[:, 512 + lo:512 + lo + 128], tri01)
                    return (j, p12, lo)

                def emit_pv(j, p12, lo):
                    last = j == nj - 1
                    nc.tensor.matmul(
                        yd1[0:65, lo:], va[:, (2 * hp) % 4, j, :],
                        p12[:, lo:512], start=(j == 0), stop=last,
                        skip_group_check=True,
                    )
                    nc.tensor.matmul(
                        yd2[0:65, lo:], va[:, (2 * hp + 1) % 4, j, :],
                        p12[:, 512 + lo:1024], start=(j == 0), stop=last,
                        skip_group_check=True,
                    )

                # software pipeline: PV lags S/exp by 2 blocks so ACT exp
                # latency stays off the PE critical path; filler units
                # (projection/out-proj pieces) keep the PE busy while ACT
                # drains the exp backlog
                pend = []
                for j in range(nj):
                    pend.append(emit_s(j))
                    if len(pend) > 2:
                        emit_pv(*pend.pop(0))
                    if pull is not None and j % FILL_EVERY == FILL_EVERY - 1:
                        pull(1)
                while pend:
                    emit_pv(*pend.pop(0))

                # normalize: 1/den (row 64), broadcast to 64 rows via a
                # K=1 ones matmul at row group (64,0) (walrus rejects
                # InstPartitionBroadcast), multiply into yt. The broadcast
                # PSUM tile borrows the s12 rotation slot.
                for head, yd in ((0, yd1), (1, yd2)):
                    r1 = rp.tile([128, 512], f32r, name="r1")
                    nc.vector.reciprocal(out=r1[64:65, :], in_=yd[64:65, :])
                    bct = ps_s.tile([128, 1024], f32, name="s12")
                    bc = bct[0:64, 0:512]
                    nc.tensor.matmul(
                        bc, ones_sb[64:65, :], r1[64:65, :],
                        start=True, stop=True,
                    )
                    rb = rbp.tile([64, 512], f32, name="rb")
                    nc.vector.tensor_copy(out=rb, in_=bc)
                    if head == 0:
                        nc.vector.tensor_mul(
                            yt[0:64, hp, bass.ts(g, 512)], yd[0:64, :], rb)
                    else:
                        ytmp = ytp.tile([64, 512], bf16, name="ytmp")
                        nc.vector.tensor_mul(ytmp, yd[0:64, :], rb)
                        nc.sync.dma_start(
                            out=yt[64:128, hp, bass.ts(g, 512)], in_=ytmp)

            def outproj(g):
                for mo in range(8):
                    op = acc_tile()
                    for m in range(2):
                        nc.tensor.matmul(
                            op, wp_sb[:, m, bass.ts(mo, 128)],
                            yt[:, m, bass.ts(g, 512)],
                            start=(m == 0), stop=(m == 1),
                        )
                    ob = obp.tile([128, 512], bf16, name="ob")
                    if mo % 2 == 0:
                        nc.vector.tensor_copy(out=ob, in_=op)
                    else:
                        nc.scalar.copy(out=ob, in_=op)
                    nc.sync.dma_start(
                        out=out_d[bass.ts(mo, 128), bass.ts(g, 512)], in_=ob)

            # ---- emission schedule ----
            # Unit queue: projection / V / out-proj pieces are emitted either
            # as prerequisites before the attention chunk that needs them or
            # pulled one at a time between attention blocks as PE filler
            # while the ACT engine works through the exp stream.
            queue = []

            def unit_q(m, g):
                def emit():
                    ts_g = bass.ts(g, 512)
                    qp = acc_tile()
                    for k in range(KT):
                        nc.tensor.matmul(
                            qp, wq_sb[:, k, bass.ts(m, 128)], xt[:, k, ts_g],
                            start=(k == 0), stop=(k == KT - 1),
                        )
                    nc.scalar.activation(
                        out=qt[:, m, ts_g], in_=qp, func=ident_fn,
                        bias=bq_sb[:, m, :], scale=1.0,
                    )
                return emit

            def unit_k(m, g):
                def emit():
                    ts_g = bass.ts(g, 512)
                    kp = acc_tile()
                    for k in range(KT):
                        nc.tensor.matmul(
                            kp, wk_sb[:, k, bass.ts(m, 128)], xt[:, k, ts_g],
                            start=(k == 0), stop=(k == KT - 1),
                        )
                    nc.scalar.copy(out=kt[:, m, ts_g], in_=kp)
                return emit

            def unit_v(it):
                def emit():
                    vp_full = acc_tile()
                    vp = vp_full[:, 0:DC]
                    for k in range(KT):
                        nc.tensor.matmul(
                            vp, xt[:, k, bass.ts(it, 128)], wv_sb[:, k, :],
                            start=(k == 0), stop=(k == KT - 1),
                        )
                    nc.vector.tensor_copy(
                        out=va[:, :, it, 0:64],
                        in_=vp.rearrange("p (h d) -> p h d", h=HC),
                    )
                return emit

            def unit_e(g, mo):
                def emit():
                    op = acc_tile()
                    for m in range(2):
                        nc.tensor.matmul(
                            op, wp_sb[:, m, bass.ts(mo, 128)],
                            yt[:, m, bass.ts(g, 512)],
                            start=(m == 0), stop=(m == 1),
                        )
                    ob = obp.tile([128, 512], bf16, name="ob")
                    if mo % 2 == 0:
                        nc.vector.tensor_copy(out=ob, in_=op)
                    else:
                        nc.scalar.copy(out=ob, in_=op)
                    nc.sync.dma_start(
                        out=out_d[bass.ts(mo, 128), bass.ts(g, 512)], in_=ob)
                return emit

            def pull(n):
                for _ in range(min(n, len(queue))):
                    queue.pop(0)()

            def drain_to(n_left):
                while len(queue) > n_left:
                    queue.pop(0)()

            # prerequisites for attn(0,0) run eagerly (PE ramps while the
            # remaining input DMAs land)
            unit_q(0, 0)()
            unit_k(0, 0)()
            for it in range(4):
                unit_v(it)()

            for g in range(1, TCH):
                queue.append(unit_q(0, g))
                queue.append(unit_k(0, g))
                for it in range(4 * g, 4 * g + 4):
                    queue.append(unit_v(it))
            for g in range(TCH):
                queue.append(unit_q(1, g))
                queue.append(unit_k(1, g))
            n_after = len(queue)

            for g in range(TCH):
                attn(0, g, pull)
                # drain prerequisites of the next attention chunk
                n_after -= 6 if g < TCH - 1 else 0
                drain_to(n_after)
            for g in range(TCH):
                drain_to(n_after - 2 * (g + 1))
                attn(1, g, pull)
                if g > 0:
                    for mo in range(8):
                        queue.append(unit_e(g - 1, mo))
            for mo in range(8):
                queue.append(unit_e(TCH - 1, mo))
            drain_to(0)

    _split_excess_waits(nc)
    return nc


def kernel(**inputs) -> np.ndarray:
    query = np.ascontiguousarray(np.asarray(inputs["query"], dtype=np.float32))
    Wq = np.asarray(inputs["Wq"], dtype=np.float32)
    Wk = np.asarray(inputs["Wk"], dtype=np.float32)
    Wv = np.asarray(inputs["Wv"], dtype=np.float32)
    Wp = np.asarray(inputs["Wp"], dtype=np.float32)
    bq = np.asarray(inputs["bq"], dtype=np.float32)
    bk = np.asarray(inputs["bk"], dtype=np.float32)  # noqa: F841 (exactly dropped)
    bv = np.asarray(inputs["bv"], dtype=np.float32)
    bp = np.asarray(inputs["bp"], dtype=np.float32)
    n_head = int(inputs.get("n_head", H))
    assert n_head == H, f"kernel hardcodes n_head={H}, got {n_head}"
    assert query.shape == (B, T, C)

    if "nc" not in _CACHE:
        _CACHE["nc"] = _build_program()
    nc = _CACHE["nc"]

    # bv contributes bv @ Wp to every output row (softmax rows sum to 1)
    bp_eff = bp + bv @ Wp

    xt_b = [np.ascontiguousarray(query[b].T).astype(BF) for b in range(B)]
    in_maps = []
    for c in range(8):
        b, hg = divmod(c, 4)
        cols = slice(DC * hg, DC * (hg + 1))
        in_maps.append({
            "xt": xt_b[b],
            "wq": np.ascontiguousarray(Wq[:, cols]).astype(BF),
            "wk": np.ascontiguousarray(Wk[:, cols]).astype(BF),
            "wv": np.ascontiguousarray(Wv[:, cols]).astype(BF),
            "wp": np.ascontiguousarray(Wp[cols, :]).astype(BF),
            "bq": np.ascontiguousarray(bq[cols]),
        })

    res = run_bass_kernel_spmd(nc, in_maps, core_ids=list(range(8)))
    _CACHE["last_res"] = res

    out = np.empty((B, T, C), np.float32)
    for b in range(B):
        acc = np.asarray(res.results[4 * b]["out_t"], dtype=np.float32)
        for c in range(4 * b + 1, 4 * b + 4):
            acc = acc + np.asarray(res.results[c]["out_t"], dtype=np.float32)
        out[b] = acc.T + bp_eff
    return out
